# revision 2
# baseline (speedup 1.0000x reference)
"""Trainium2 Bass kernel for the 2-qubit quantum-circuit batch evaluation.

Reference semantics (per batch row, x = [x0, x1], scalar theta):
    state = RY(theta) @ CNOT @ (RY(x0)|0> (x) RY(x1)|0>)
    out = (<Z> + 1)/2 for each qubit.

Algebraically (product/half-angle identities):
    out0 = 0.5 + 0.5*cos(th)*cos(x0) - 0.5*sin(th)*sin(x0)*sin(x1)
    out1 = 0.5 + 0.5*cos(x0)*cos(x1)

Rewritten so every trig evaluation is a *squared* sine (sign-free, and the
pair angles need no extra range reduction once x itself is reduced):
    y  = x - 2*pi*round(x/(2*pi))          in [-pi, pi]
    qu = sin^2((y0+y1)/2),  qv = sin^2((y0-y1)/2),  q0 = sin^2(y0/2)
    cos(x0)        = 1 - 2*q0
    cos(x0)cos(x1) = 1 - qu - qv
    sin(x0)sin(x1) = qu - qv
so with A = 0.5+0.5cos(th), C = cos(th), D = 0.5sin(th):
    out0 = (A - C*q0) - D*(qu - qv)
    out1 = 1 - 0.5*(qu + qv)

Engine split (measured costs, per full pass over a tile):
  Pool/GPSIMD: the two magic-rounding tensor_scalars (ts runs ~full rate on
      Pool) + the -D scale;  ACT: the three Sins + the pair Square;
  DVE: the f16 tensor_tensors (2x mode) + two 4x f16 tensor_scalars.
fp16 end-to-end is safe: tolerance is 2e-2, numpy-simulated worst error
(incl. fp16 k) is 5.5e-3; HW Sin is accurate to |arg| <= ~3.35.

Sharding: pure data parallel over 8 cores. Host dein terleaves x into packed
x0/x1 halves (fp16) and re-interleaves the fp16 outputs (layout + dtype work
only). theta-derived scalars ride a tiny [128, 4] constant tensor.
"""

import numpy as np

import concourse.bass as bass
import concourse.mybir as mybir
from concourse.alu_op_type import AluOpType
from concourse.bacc import Bacc
from concourse.tile import TileContext
from concourse import bass_utils

N_CORES = 8
B = 8388608
BC = B // N_CORES            # rows per core
P = 128                      # SBUF partitions
F = 4096                     # free elems per partition per tile (x0-half | x1-half)
H = F // 2
T = (BC * 2) // (P * F)      # tiles per core
MAGIC = float(1.5 * 2**23)   # f32 round-to-nearest-int magic constant
INV2PI = float(1.0 / (2.0 * np.pi))
NEG2PI = float(-2.0 * np.pi)

_CACHE = {}


def _build_nc():
    nc = Bacc()
    f32 = mybir.dt.float32
    f16 = mybir.dt.float16
    x = nc.dram_tensor("x", [T, P, F], f16, kind="ExternalInput")
    consts = nc.dram_tensor("consts", [P, 4], f32, kind="ExternalInput")
    out = nc.dram_tensor("out", [T, P, F], f16, kind="ExternalOutput")

    Sin = mybir.ActivationFunctionType.Sin
    Square = mybir.ActivationFunctionType.Square

    with TileContext(nc) as tc:
        with tc.tile_pool(name="cpool", bufs=1) as cpool, \
             tc.tile_pool(name="io", bufs=2) as io, \
             tc.tile_pool(name="mid", bufs=2) as mid, \
             tc.tile_pool(name="tail", bufs=2) as tail:
            ct = cpool.tile([P, 4], f32)
            nc.sync.dma_start(out=ct[:], in_=consts[:])
            negC = ct[:, 0:1]   # -cos(theta)
            Aap = ct[:, 1:2]    # 0.5 + 0.5*cos(theta)
            negD = ct[:, 2:3]   # -0.5*sin(theta)

            for i in range(T):
                xt = io.tile([P, F], f16, tag="xt")
                nc.sync.dma_start(out=xt[:], in_=x[i])

                # --- range reduction (Pool does the two ts, DVE the f16 add)
                t32 = mid.tile([P, F], f32, tag="t32")
                k16 = mid.tile([P, F], f16, tag="k16")
                y16 = mid.tile([P, F], f16, tag="y16")
                nc.gpsimd.tensor_scalar(
                    t32[:], xt[:], INV2PI, MAGIC, AluOpType.mult, AluOpType.add,
                )
                nc.gpsimd.tensor_scalar(
                    k16[:], t32[:], MAGIC, NEG2PI, AluOpType.subtract, AluOpType.mult,
                )
                nc.vector.tensor_tensor(y16[:], xt[:], k16[:], AluOpType.add)
                y0 = y16[:, 0:H]
                y1 = y16[:, H:F]

                # --- pair angles yu = y0+y1, yv = y0-y1 (|.| <= 2pi)
                yuv = mid.tile([P, F], f16, tag="yuv")
                nc.vector.tensor_tensor(yuv[:, 0:H], y0, y1, AluOpType.add)
                nc.vector.tensor_tensor(yuv[:, H:F], y0, y1, AluOpType.subtract)

                # --- sines of half-angles (ACT), squares
                ss = mid.tile([P, F + H], f16, tag="ss")
                nc.scalar.activation(ss[:, 0:F], yuv[:], Sin, scale=0.5)
                nc.scalar.activation(ss[:, F:], y0, Sin, scale=0.5)
                qq = mid.tile([P, F + H], f16, tag="qq")
                nc.scalar.activation(qq[:, 0:F], ss[:, 0:F], Square)
                nc.vector.tensor_tensor(
                    qq[:, F:], ss[:, F:], ss[:, F:], AluOpType.mult
                )
                qu = qq[:, 0:H]
                qv = qq[:, H:F]
                q0 = qq[:, F:]

                # --- combines
                ot = io.tile([P, F], f16, tag="ot")
                a16 = tail.tile([P, H], f16, tag="a16")
                g16 = tail.tile([P, H], f16, tag="g16")
                gp16 = tail.tile([P, H], f16, tag="gp16")
                p16 = tail.tile([P, H], f16, tag="p16")
                # a = A - C*q0          (DVE ts @4x, runtime per-partition scalars)
                nc.vector.tensor_scalar(
                    a16[:], q0, negC, Aap, AluOpType.mult, AluOpType.add,
                )
                # g = qu - qv ; gp = -D*g (Pool) ; out0 = gp + a
                nc.vector.tensor_tensor(g16[:], qu, qv, AluOpType.subtract)
                nc.gpsimd.tensor_scalar(
                    gp16[:], g16[:], negD, None, AluOpType.mult,
                )
                nc.vector.tensor_tensor(ot[:, 0:H], gp16[:], a16[:], AluOpType.add)
                # p = qu + qv ; out1 = 1 - 0.5*p (DVE ts @4x)
                nc.vector.tensor_tensor(p16[:], qu, qv, AluOpType.add)
                nc.vector.tensor_scalar(
                    ot[:, H:F], p16[:], -0.5, 1.0, AluOpType.mult, AluOpType.add,
                )

                nc.sync.dma_start(out=out[i], in_=ot[:])
    nc.compile()
    return nc


def _run(in_maps, trace=False, trace_cores=None):
    if "nc" not in _CACHE:
        _CACHE["nc"] = _build_nc()
    return bass_utils.run_bass_kernel_spmd(
        _CACHE["nc"],
        in_maps,
        core_ids=list(range(N_CORES)),
        trace=trace,
        trace_cores=trace_cores,
    )


def kernel(x, theta, _trace=False, _trace_cores=None):
    x = np.asarray(x)
    theta = np.asarray(theta, dtype=np.float32)
    assert x.shape == (B, 2), x.shape

    th = float(theta.reshape(-1)[0])
    consts = np.empty((P, 4), dtype=np.float32)
    consts[:, 0] = -np.cos(th)
    consts[:, 1] = 0.5 + 0.5 * np.cos(th)
    consts[:, 2] = -0.5 * np.sin(th)
    consts[:, 3] = 0.0

    # host layout: per core, [T, P, F] fp16 with x0-half | x1-half per tile
    x16 = x.astype(np.float16)                       # [B, 2]
    x0 = x16[:, 0].reshape(N_CORES, T, P, H)
    x1 = x16[:, 1].reshape(N_CORES, T, P, H)
    xs = np.concatenate([x0, x1], axis=3)            # [N, T, P, F]
    xs = np.ascontiguousarray(xs)

    in_maps = [{"x": xs[c], "consts": consts} for c in range(N_CORES)]
    res = _run(in_maps, trace=_trace, trace_cores=_trace_cores)
    _CACHE["last_results"] = res

    o = np.stack([res.results[c]["out"] for c in range(N_CORES)])  # [N,T,P,F] f16
    out = np.empty((B, 2), dtype=np.float32)
    out[:, 0] = o[:, :, :, 0:H].reshape(B).astype(np.float32)
    out[:, 1] = o[:, :, :, H:F].reshape(B).astype(np.float32)
    return out
